# revision 13
# baseline (speedup 1.0000x reference)
"""BiDAF attention-flow kernel for Trainium2 (8 NeuronCores, data-parallel).

Self-contained: hardcodes shapes B,C,Q,H2 = 64,512,64,256; n_labels=2.
kernel(**inputs) takes full unsharded inputs, shards batch over 8 cores,
runs one SPMD Bass/Tile kernel, gathers [8,2] per core -> [64,2].

Per-core math (8 examples, bf16 compute, fp32 accumulation):
  S = c @ diag(w_m) @ q^T + (c@w_c)[:,None] + (q@w_q)[None,:]
    - the c@w_c term folds into the matmul rhs (rhs = w_m*q^T + w_c),
    - the q@w_q term rides in via a K=1 all-ones broadcast matmul.
  P = exp(S) unstabilized (|S| is O(1) for this distribution); row-softmax
  needs only row sums, and b_att = softmax(max_j S) = Pmax/sum(Pmax) with
  Pmax = max_j P (exp is monotone).

Structure vs the previous version:
  - c loads: one SWDGE cast-DMA per example, unchained (same-ring in-order
    drain gives streaming completion); weights ride the sync-queue HWDGE
    (wsim flat + on-chip PE transpose), wlab issued last (needed at end).
  - max-pools via fused tensor_tensor_reduce (fold+reduce in one DVE op).
  - q2c flipped: lhsT=c chunk, rhs=pm column -> q2c^T lands as a PSUM
    column in the right (d-major) layout, no epilogue transposes.
  - software-pipelined issue order: pair p+1's c^T transposes are issued
    between S(p) and P^T(p) so the PE stays busy during softmax latency.
"""

import os
import sys

for _p in ("/opt/trn_rl_repo", "/opt/pypackages"):
    if os.path.isdir(_p) and _p not in sys.path:
        sys.path.insert(0, _p)

import numpy as np

import concourse.bass as bass
import concourse.bacc as bacc
import concourse.tile as tile
import concourse.mybir as mybir
from concourse.bass_utils import run_bass_kernel_spmd
from concourse.masks import make_identity
from concourse.tile_rust import add_dep_helper

F32 = mybir.dt.float32
BF16 = mybir.dt.bfloat16
AX = mybir.AxisListType
OP = mybir.AluOpType
AF = mybir.ActivationFunctionType

N_CORES = 8
B, C, Q, H2 = 64, 512, 64, 256
NL = 2
EX = B // N_CORES          # examples per core = 8
CH = C // 128              # context chunks of 128 = 4
DH = H2 // 128             # feature chunks of 128 = 2
NK = 4 * DH                # final feature chunks (4 pieces x DH) = 8
NEG = -1e30
POS = 1e30

USE_TTR = int(os.environ.get("K_USE_TTR", "1"))  # 0=fold trees, 1=ttr max, 2=ttr min(-x)
CHAIN_LOADS = os.environ.get("K_CHAIN_LOADS", "0") == "1"


def _body(tc, ctx, fd, fq, wsim, wlab, blab, out):
    nc = tc.nc

    consts = ctx.enter_context(tc.tile_pool(name="consts", bufs=1))
    bigbuf = ctx.enter_context(tc.tile_pool(name="bigbuf", bufs=1))
    sbp = ctx.enter_context(tc.tile_pool(name="sbp", bufs=1))
    c2_pool = ctx.enter_context(tc.tile_pool(name="c2", bufs=2))
    scr_pool = ctx.enter_context(tc.tile_pool(name="scr", bufs=2))

    ps_ct = ctx.enter_context(tc.tile_pool(name="psct", bufs=2, space="PSUM"))
    ps_s = ctx.enter_context(tc.tile_pool(name="pss", bufs=2, space="PSUM"))
    ps_c2q = ctx.enter_context(tc.tile_pool(name="psc2q", bufs=2, space="PSUM"))

    # ---- small constants ----
    ones_bf = consts.tile([1, 128], BF16)      # K=1 broadcast lhsT
    nc.vector.memset(ones_bf[0:1, :], 1.0)
    ones128_bf = consts.tile([128, 1], BF16)   # partition-sum lhsT
    nc.vector.memset(ones128_bf[:, :], 1.0)
    ones_f32 = consts.tile([1, 128], F32)      # broadcast lhsT + [1,1] identity
    nc.vector.memset(ones_f32[0:1, :], 1.0)
    id_bf = consts.tile([128, 128], BF16)      # identity for PE transposes
    make_identity(nc, id_bf[:, :])

    # ---- weights: wsim/blab on the sync HWDGE queue (keeps gpsimd free) ----
    wsim_flat = consts.tile([1, 3 * H2], F32)
    nc.sync.dma_start(wsim_flat[0:1, :], wsim[:].rearrange("(o x) -> o x", o=1))
    b_sb = consts.tile([1, NL], F32)
    nc.sync.dma_start(b_sb[0:1, :], blab[:].rearrange("(o l) -> o l", o=1))

    # ---- big inputs: cast-load fp32 -> bf16 (SWDGE), unchained per example.
    # All loads share SWDGE ring 0; each engine drains its ring FIFO, so
    # earlier dma_starts complete first and compute streams behind the loads.
    q_dup = bigbuf.tile([128, EX, H2], BF16)        # q on both 64-partition halves
    nc.gpsimd.dma_start(q_dup[0:64, :, :], fq[:, :, :].rearrange("e j d -> j e d"))
    nc.gpsimd.dma_start(q_dup[64:128, :, :], fq[:, :, :].rearrange("e j d -> j e d"))
    c_nat = bigbuf.tile([128, EX, CH, H2], BF16)   # p = i%128
    prev = None
    for e in range(EX):
        ld = nc.gpsimd.dma_start(
            c_nat[:, e, :, :],
            fd[e:e + 1, :, :].rearrange("e (ch p) d -> p (e ch) d", p=128),
        )
        if CHAIN_LOADS and prev is not None:
            add_dep_helper(ld.ins, prev.ins, sync=True, reason="load chain")
        prev = ld
    # wlab: strided SWDGE load (8B granules), only needed at the very end
    wlab_sb = consts.tile([128, NK, NL], F32)  # chunk k = piece*DH+dh
    nc.gpsimd.dma_start(wlab_sb[:, :, :], wlab[:, :].rearrange("(k p) l -> p k l", p=128))

    # ---- persistent SBUF state ----
    c_T = bigbuf.tile([128, EX, DH, C], BF16)
    q_T = sbp.tile([128, EX, DH, Q], BF16)
    rhs_qm = sbp.tile([128, EX, DH, Q], BF16)
    qwrow = sbp.tile([1, EX * Q], BF16)
    w_sb = consts.tile([128, 6], F32)          # col = t*2+dh; t: 0=w_c 1=w_q 2=w_m
    wq_bf = consts.tile([128, DH], BF16)
    P_all = sbp.tile([128, CH, EX, Q], BF16)
    Pn_all = sbp.tile([128, CH, EX, Q], BF16)
    PT_all = sbp.tile([128, EX // 2, CH, 128], BF16)
    pm_col = sbp.tile([128, EX * CH], BF16)    # col = e*CH+ch
    den_all = sbp.tile([128, CH, EX], F32)
    rden_all = sbp.tile([128, CH, EX], F32)
    p0_f = sbp.tile([128, EX * DH], F32)       # cmax   (col = e*DH+dh)
    p1_f = sbp.tile([128, EX * DH], F32)       # max c2q
    p2_f = sbp.tile([128, EX * DH], F32)       # max c*c2q
    p3_f = sbp.tile([128, EX * DH], F32)       # max c*q2c
    cmin_f = sbp.tile([128, EX * DH], F32)
    q2cT_sb = sbp.tile([128, EX * DH], F32)    # unnormalized q2c^T columns
    q2cr_sb = sbp.tile([128, EX * DH], F32)
    sumb = sbp.tile([1, EX], F32)
    recipb = sbp.tile([1, EX], F32)
    r_sb = sbp.tile([128, EX], F32)
    out_sb = sbp.tile([EX, NL], F32)

    def pe_group(dsts, srcs, f32_id=False):
        """One PSUM transpose accumulation group (start first, stop last)."""
        ident = ones_f32 if f32_id else id_bf
        first = None
        n = len(srcs)
        for k, src in enumerate(srcs):
            mm = nc.tensor.matmul(
                dsts[k], src, ident[0:src.shape[0], 0:src.shape[0]],
                is_transpose=True,
                start=(first is None), stop=(k == n - 1),
                skip_group_check=True,
            )
            if first is None:
                first = mm
            else:
                add_dep_helper(mm.ins, first.ins, sync=False, reason="bank order")
        return first

    # ---- w_sb: transpose wsim_flat [1,768] into columns [128, 6] ----
    w_ps = ps_s.tile([128, 512], F32, tag="s")
    pe_group(
        [w_ps[:, t:t + 1] for t in range(6)],
        [wsim_flat[0:1, t * 128:(t + 1) * 128] for t in range(6)],
        f32_id=True,
    )
    nc.scalar.copy(w_sb[:, :], w_ps[:, 0:6])
    nc.vector.tensor_copy(wq_bf[:, :], w_sb[:, 2:4])

    # ---- q^T for all examples + rhs_qm + qw rows ----
    for half in range(2):
        tp = ps_ct.tile([128, DH, CH, 128], BF16, tag="ct")
        tpv = tp[:, :, :, :].rearrange("p a b x -> p (a b x)")
        srcs = []
        for e in range(half * 4, half * 4 + 4):
            for dh in range(DH):
                srcs.append(q_dup[0:64, e, dh * 128:(dh + 1) * 128])
        pe_group([tpv[:, k * Q:(k + 1) * Q] for k in range(8)], srcs)
        nc.scalar.copy(
            q_T[:, half * 4:half * 4 + 4, :, :],
            tpv[:, 0:8 * Q].rearrange("p (e dh j) -> p e dh j", dh=DH, j=Q),
        )
    for dh in range(DH):
        nc.scalar.activation(
            rhs_qm[:, :, dh, :], q_T[:, :, dh, :],
            AF.Identity,
            bias=w_sb[:, 0 + dh:1 + dh], scale=w_sb[:, 4 + dh:5 + dh],
        )
    qw_ps = ps_s.tile([128, 512], F32, tag="s")
    first = None
    for e in range(EX):
        for dh in range(DH):
            mm = nc.tensor.matmul(
                qw_ps[0:1, e * Q:(e + 1) * Q], wq_bf[:, dh:dh + 1], q_T[:, e, dh, :],
                start=(dh == 0), stop=(dh == DH - 1),
                skip_group_check=True,
            )
            if first is None:
                first = mm
            else:
                add_dep_helper(mm.ins, first.ins, sync=False, reason="bank order")
    nc.vector.tensor_copy(qwrow[0:1, :], qw_ps[0:1, 0:EX * Q])

    # ---------- pipelined per-pair stages ----------
    def stage_T(p):
        """c^T transposes for pair p (PE) + per-example ACT evacuation."""
        for e in (2 * p, 2 * p + 1):
            tp = ps_ct.tile([128, DH, CH, 128], BF16, tag="ct")
            for dh in range(DH):
                pe_group(
                    [tp[:, dh, chk, :] for chk in range(CH)],
                    [c_nat[:, e, chk, dh * 128:(dh + 1) * 128] for chk in range(CH)],
                )
            nc.scalar.copy(
                c_T[:, e, :, :],
                tp[:, :, :, :].rearrange("p dh ch x -> p dh (ch x)"),
            )

    def stage_S(p):
        """S matmuls (PE) -> exp (ACT) -> den/pm/recip/Pn (DVE)."""
        e0 = 2 * p
        ps = ps_s.tile([128, CH, 2, Q], F32, tag="s")
        first = None
        for slot in range(2):
            e = e0 + slot
            for chk in range(CH):
                for dh in range(DH):
                    mm = nc.tensor.matmul(
                        ps[:, chk, slot, :],
                        c_T[:, e, dh, chk * 128:(chk + 1) * 128],
                        rhs_qm[:, e, dh, :],
                        start=(first is None), stop=False,
                        skip_group_check=True,
                    )
                    if first is None:
                        first = mm
                    else:
                        add_dep_helper(mm.ins, first.ins, sync=False, reason="bank")
                mm = nc.tensor.matmul(
                    ps[:, chk, slot, :], ones_bf[0:1, :],
                    qwrow[0:1, e * Q:(e + 1) * Q],
                    start=False, stop=(slot == 1 and chk == CH - 1),
                    skip_group_check=True,
                )
                add_dep_helper(mm.ins, first.ins, sync=False, reason="bank")

        pview = P_all[:, :, e0:e0 + 2, :]
        nc.scalar.activation(pview, ps[:, :, :, :], AF.Exp)
        den = den_all[:, :, e0:e0 + 2]
        nc.vector.reduce_sum(den, pview, axis=AX.X)
        nc.vector.tensor_reduce(
            pm_col[:, e0 * CH:(e0 + 2) * CH].rearrange("p (e c) -> p c e", c=CH),
            pview, axis=AX.X, op=OP.max,
        )
        rden = rden_all[:, :, e0:e0 + 2]
        nc.vector.reciprocal(rden, den)
        nc.vector.tensor_tensor(
            Pn_all[:, :, e0:e0 + 2, :], pview,
            rden.unsqueeze(3).broadcast_to([128, CH, 2, Q]),
            op=OP.mult,
        )

    def stage_PT(p):
        """P_norm^T via PE transpose + ACT evacuation."""
        e0 = 2 * p
        tp = ps_s.tile([128, CH, 128], BF16, tag="s")
        pe_group(
            [tp[:, chk, :] for chk in range(CH)],
            [Pn_all[:, chk, e0:e0 + 2, :] for chk in range(CH)],
        )
        nc.scalar.copy(PT_all[:, p, :, :], tp[:, :, :])

    def stage_C2Q(p):
        """c2q^T = q^T @ P^T per (slot, dh), ACT evacuation to bf16."""
        c2q_sb = c2_pool.tile([128, 2, DH, C], BF16, tag="c2q")
        for slot in range(2):
            e = 2 * p + slot
            for dh in range(DH):
                ps = ps_c2q.tile([128, C], F32, tag="c2q")
                nc.tensor.matmul(
                    ps[:, :],
                    q_dup[slot * 64:slot * 64 + 64, e, dh * 128:(dh + 1) * 128],
                    PT_all[slot * 64:slot * 64 + 64, p, :, :],
                    start=True, stop=True,
                    tile_position=(slot * 64, 0),
                )
                nc.scalar.copy(c2q_sb[:, slot, dh, :], ps[:, :])
        return c2q_sb

    def stage_Q2C(p):
        """q2c^T columns: lhsT=c chunk, rhs=pm column; DVE evacuation."""
        ps = ps_s.tile([128, 2, DH], F32, tag="s")
        for slot in range(2):
            e = 2 * p + slot
            for dh in range(DH):
                first = None
                for chk in range(CH):
                    mm = nc.tensor.matmul(
                        ps[:, slot, dh:dh + 1],
                        c_nat[:, e, chk, dh * 128:(dh + 1) * 128],
                        pm_col[:, e * CH + chk:e * CH + chk + 1],
                        start=(chk == 0), stop=(chk == CH - 1),
                        skip_group_check=True,
                    )
                    if first is None:
                        first = mm
                    else:
                        add_dep_helper(mm.ins, first.ins, sync=False, reason="grp")
        nc.vector.tensor_copy(
            q2cT_sb[:, 2 * p * DH:(2 * p + 2) * DH].rearrange(
                "p (s dh) -> p s dh", dh=DH),
            ps[:, :, :],
        )

    def stage_FOLD(p, c2q_sb):
        """Fused fold+reduce max-pools over context (DVE ttr), or fold trees."""
        if USE_TTR:
            # USE_TTR==2: reduce stage only supports min -> compute
            # min(-(x)) = -max(x) via scale=-1; epilogue negates pieces.
            sgn = 1.0 if USE_TTR == 1 else -1.0
            mx = OP.max if USE_TTR == 1 else OP.min
            init = NEG if USE_TTR == 1 else POS
            for slot in range(2):
                e = 2 * p + slot
                for dh in range(DH):
                    col = e * DH + dh
                    scrA = scr_pool.tile([128, 256], BF16, tag="scrA")
                    nc.vector.tensor_tensor_reduce(
                        scrA[:, :],
                        c_T[:, e, dh, 0:256], c_T[:, e, dh, 256:512],
                        sgn, init, OP.max, mx,
                        p0_f[:, col:col + 1],
                    )
                    scrB = scr_pool.tile([128, 256], BF16, tag="scrB")
                    nc.vector.tensor_tensor_reduce(
                        scrB[:, :],
                        c_T[:, e, dh, 0:256], c_T[:, e, dh, 256:512],
                        1.0, POS, OP.min, OP.min,
                        cmin_f[:, col:col + 1],
                    )
                    scrC = scr_pool.tile([128, 256], BF16, tag="scrC")
                    nc.vector.tensor_tensor_reduce(
                        scrC[:, :],
                        c2q_sb[:, slot, dh, 0:256], c2q_sb[:, slot, dh, 256:512],
                        sgn, init, OP.max, mx,
                        p1_f[:, col:col + 1],
                    )
                    scrD = scr_pool.tile([128, 512], BF16, tag="scrD")
                    nc.vector.tensor_tensor_reduce(
                        scrD[:, :],
                        c_T[:, e, dh, :], c2q_sb[:, slot, dh, :],
                        sgn, init, OP.mult, mx,
                        p2_f[:, col:col + 1],
                    )
            return
        # fallback: 2x-mode tensor_tensor fold trees + short reduces
        eP = slice(2 * p, 2 * p + 2)
        pview = lambda t, col0: t[:, col0:col0 + 2 * DH].rearrange(
            "p (e dh) -> p e dh", dh=DH)
        col0 = 2 * p * DH
        fA = scr_pool.tile([128, 2, DH, 256], BF16, tag="fA")
        fB = scr_pool.tile([128, 2, DH, 128], BF16, tag="fB")
        # p1: max c2q
        nc.vector.tensor_tensor(
            fA[:, :, :, :], c2q_sb[:, :, :, 0:256], c2q_sb[:, :, :, 256:512], op=OP.max)
        nc.vector.tensor_tensor(
            fA[:, :, :, 0:128], fA[:, :, :, 0:128], fA[:, :, :, 128:256], op=OP.max)
        nc.vector.tensor_reduce(
            pview(p1_f, col0), fA[:, :, :, 0:128], axis=AX.X, op=OP.max)
        # p2: max c*c2q
        prod = scr_pool.tile([128, 2, DH, C], BF16, tag="prod")
        nc.vector.tensor_tensor(
            prod[:, :, :, :], c_T[:, eP, :, :], c2q_sb[:, :, :, :], op=OP.mult)
        nc.vector.tensor_tensor(
            fA[:, :, :, :], prod[:, :, :, 0:256], prod[:, :, :, 256:512], op=OP.max)
        nc.vector.tensor_tensor(
            fB[:, :, :, :], fA[:, :, :, 0:128], fA[:, :, :, 128:256], op=OP.max)
        nc.vector.tensor_reduce(
            pview(p2_f, col0), fB[:, :, :, :], axis=AX.X, op=OP.max)
        # p0 / cmin on c
        for op, dst in ((OP.max, pview(p0_f, col0)), (OP.min, pview(cmin_f, col0))):
            nc.vector.tensor_tensor(
                fA[:, :, :, :], c_T[:, eP, :, 0:256], c_T[:, eP, :, 256:512], op=op)
            nc.vector.tensor_tensor(
                fB[:, :, :, :], fA[:, :, :, 0:128], fA[:, :, :, 128:256], op=op)
            nc.vector.tensor_reduce(dst, fB[:, :, :, :], axis=AX.X, op=op)

    # ---------- run the pipeline ----------
    stage_T(0)
    stage_S(0)
    for p in range(EX // 2):
        if p + 1 < EX // 2:
            stage_T(p + 1)
        stage_PT(p)
        c2q_sb = stage_C2Q(p)
        stage_Q2C(p)
        stage_FOLD(p, c2q_sb)
        if p + 1 < EX // 2:
            stage_S(p + 1)

    # ---------- epilogue: b_att sums, q2c scale, piece3, final matmul ----------
    if USE_TTR == 2:
        # min(-x) reductions left the max pieces negated
        for t in (p0_f, p1_f, p2_f):
            nc.vector.tensor_scalar_mul(t[:, :], t[:, :], -1.0)
    bs_ps = ps_s.tile([128, 512], F32, tag="s")
    nc.tensor.matmul(
        bs_ps[0:1, 0:EX * CH], ones128_bf[:, :], pm_col[:, :],
        start=True, stop=True,
    )
    nc.vector.reduce_sum(
        sumb[0:1, :],
        bs_ps[0:1, 0:EX * CH].rearrange("o (e c) -> o e c", c=CH),
        axis=AX.X,
    )
    nc.vector.reciprocal(recipb[0:1, :], sumb[0:1, :])
    rb_ps = ps_s.tile([128, 512], F32, tag="s")
    nc.tensor.matmul(
        rb_ps[:, 0:EX], ones_f32[0:1, :], recipb[0:1, :],
        start=True, stop=True,
    )
    nc.vector.tensor_copy(r_sb[:, :], rb_ps[:, 0:EX])

    q2v = q2cT_sb[:, :].rearrange("p (e dh) -> p e dh", dh=DH)
    nc.vector.tensor_tensor(
        q2cr_sb[:, :].rearrange("p (e dh) -> p e dh", dh=DH),
        q2v, r_sb[:, :].unsqueeze(2).broadcast_to([128, EX, DH]), op=OP.mult)
    s3a = scr_pool.tile([128, EX * DH], F32, tag="s3a")
    s3b = scr_pool.tile([128, EX * DH], F32, tag="s3b")
    nc.vector.tensor_tensor(s3a[:, :], q2cr_sb[:, :], p0_f[:, :], op=OP.mult)
    nc.vector.tensor_tensor(s3b[:, :], q2cr_sb[:, :], cmin_f[:, :], op=OP.mult)
    nc.vector.tensor_tensor(p3_f[:, :], s3a[:, :], s3b[:, :], op=OP.max)

    out_ps = ps_s.tile([128, 512], F32, tag="s")
    pieces = [p0_f, p1_f, p2_f, p3_f]
    first = None
    for piece in range(4):
        for dh in range(DH):
            pv = pieces[piece][:, :].rearrange("p (e d) -> p e d", d=DH)[:, :, dh]
            mm = nc.tensor.matmul(
                out_ps[0:EX, 0:NL], pv, wlab_sb[:, piece * DH + dh, :],
                start=(first is None), stop=False, skip_group_check=True,
            )
            if first is None:
                first = mm
            else:
                add_dep_helper(mm.ins, first.ins, sync=False, reason="bank")
    mm = nc.tensor.matmul(
        out_ps[0:EX, 0:NL], ones_f32[0:1, 0:EX], b_sb[0:1, :],
        start=False, stop=True, skip_group_check=True,
    )
    add_dep_helper(mm.ins, first.ins, sync=False, reason="bank")
    nc.vector.tensor_copy(out_sb[:, :], out_ps[0:EX, 0:NL])
    nc.sync.dma_start(out[:, :], out_sb[:, :])


def build_nc():
    nc = bacc.Bacc("TRN2", target_bir_lowering=False, debug=False)
    fd = nc.dram_tensor("fd", [EX, C, H2], F32, kind="ExternalInput")
    fq = nc.dram_tensor("fq", [EX, Q, H2], F32, kind="ExternalInput")
    wsim = nc.dram_tensor("wsim", [3 * H2], F32, kind="ExternalInput")
    wlab = nc.dram_tensor("wlab", [4 * H2, NL], F32, kind="ExternalInput")
    blab = nc.dram_tensor("blab", [NL], F32, kind="ExternalInput")
    out = nc.dram_tensor("out", [EX, NL], F32, kind="ExternalOutput")

    from contextlib import ExitStack
    with tile.TileContext(nc) as tc:
        with ExitStack() as ctx:
            _body(tc, ctx, fd[:, :, :], fq[:, :, :], wsim[:], wlab[:, :], blab[:], out[:, :])
    nc.compile()
    return nc


_NC_CACHE = None


def run(inputs, trace=False):
    global _NC_CACHE
    if _NC_CACHE is None:
        _NC_CACHE = build_nc()
    nc = _NC_CACHE

    fd = np.ascontiguousarray(np.asarray(inputs["feature_document"], dtype=np.float32))
    fq = np.ascontiguousarray(np.asarray(inputs["feature_query"], dtype=np.float32))
    wsim = np.ascontiguousarray(np.asarray(inputs["w_sim"], dtype=np.float32))
    wlab = np.ascontiguousarray(np.asarray(inputs["w_label"], dtype=np.float32))
    blab = np.ascontiguousarray(np.asarray(inputs["b_label"], dtype=np.float32))

    in_maps = []
    for core in range(N_CORES):
        sl = slice(core * EX, (core + 1) * EX)
        in_maps.append({
            "fd": fd[sl], "fq": fq[sl],
            "wsim": wsim, "wlab": wlab, "blab": blab,
        })
    res = run_bass_kernel_spmd(nc, in_maps, list(range(N_CORES)), trace=trace)
    outs = np.concatenate([np.asarray(res.results[i]["out"]) for i in range(N_CORES)], axis=0)
    return outs.astype(np.float32), res


def kernel(**inputs):
    outs, _ = run(inputs, trace=False)
    return outs


# revision 31
# speedup vs baseline: 1.2265x; 1.2265x over previous
"""BiDAF attention-flow kernel for Trainium2 (8 NeuronCores, data-parallel).

Self-contained: hardcodes shapes B,C,Q,H2 = 64,512,64,256; n_labels=2.
kernel(**inputs) takes full unsharded inputs, shards batch over 8 cores,
runs one SPMD Bass/Tile kernel, gathers [8,2] per core -> [64,2].

Per-core math (8 examples, bf16 compute, fp32 accumulation):
  S = c @ diag(w_m) @ q^T + (c@w_c)[:,None] + (q@w_q)[None,:]
    - the c@w_c term folds into the matmul rhs (rhs = w_m*q^T + w_c),
    - the q@w_q term rides in via a K=1 all-ones broadcast matmul.
  P = exp(S) unstabilized (|S| is O(1) for this distribution); row-softmax
  needs only row sums, and b_att = softmax(max_j S) = Pmax/sum(Pmax) with
  Pmax = max_j P (exp is monotone).

Structure vs the previous version:
  - c loads: one SWDGE cast-DMA per example, chained with lookahead-2
    (two DMAs in flight: no inter-link bubbles, staggered completion);
    weights ride the sync-queue HWDGE (wsim flat + on-chip PE transpose),
    wlab issued last (needed at end); q duplicated via sync SBUF copy.
  - max-pools as fold trees split across DVE and GPSIMD (GPSIMD is idle
    during compute), with one stacked tensor_reduce per pair.
  - q2c flipped: lhsT=c chunk, rhs=pm column -> q2c^T lands as a PSUM
    column in the right (d-major) layout, no epilogue transposes.
  - software-pipelined issue order: pair p+1's c^T transposes are issued
    between S(p) and P^T(p) so the PE stays busy during softmax latency;
    next pair's softmax is issued before this pair's folds on the DVE.
"""

import os
import sys

for _p in ("/opt/trn_rl_repo", "/opt/pypackages"):
    if os.path.isdir(_p) and _p not in sys.path:
        sys.path.insert(0, _p)

import numpy as np

import concourse.bass as bass
import concourse.bacc as bacc
import concourse.tile as tile
import concourse.mybir as mybir
from concourse.bass_utils import run_bass_kernel_spmd
from concourse.masks import make_identity
from concourse.tile_rust import add_dep_helper

F32 = mybir.dt.float32
BF16 = mybir.dt.bfloat16
AX = mybir.AxisListType
OP = mybir.AluOpType
AF = mybir.ActivationFunctionType

N_CORES = 8
B, C, Q, H2 = 64, 512, 64, 256
NL = 2
EX = B // N_CORES          # examples per core = 8
CH = C // 128              # context chunks of 128 = 4
DH = H2 // 128             # feature chunks of 128 = 2
NK = 4 * DH                # final feature chunks (4 pieces x DH) = 8
NEG = -1e30
POS = 1e30

USE_TTR = int(os.environ.get("K_USE_TTR", "0"))  # 0=fold trees, 1=ttr max, 2=ttr min(-x)
CHAIN_LOADS = os.environ.get("K_CHAIN_LOADS", "0") == "1"


def _body(tc, ctx, fd, fq, wsim, wlab, blab, out):
    nc = tc.nc

    consts = ctx.enter_context(tc.tile_pool(name="consts", bufs=1))
    bigbuf = ctx.enter_context(tc.tile_pool(name="bigbuf", bufs=1))
    sbp = ctx.enter_context(tc.tile_pool(name="sbp", bufs=1))
    c2_pool = ctx.enter_context(tc.tile_pool(name="c2", bufs=2))
    scr_pool = ctx.enter_context(tc.tile_pool(name="scr", bufs=2))

    ps_ct = ctx.enter_context(tc.tile_pool(name="psct", bufs=2, space="PSUM"))
    ps_s = ctx.enter_context(tc.tile_pool(name="pss", bufs=2, space="PSUM"))
    ps_c2q = ctx.enter_context(tc.tile_pool(name="psc2q", bufs=2, space="PSUM"))

    # ---- big inputs FIRST: cast-load fp32 -> bf16 (SWDGE), unchained per
    # example.  All loads share SWDGE ring 0; each engine drains its ring
    # FIFO, so earlier dma_starts complete first and compute streams behind
    # the loads.  Issued before everything else so descriptor gen starts at
    # preamble end.
    # Chain with lookahead-2: dma k waits on dma k-2, so exactly two DMAs
    # share the SDMA ring at any time -> no inter-link bubbles, but
    # completion still staggers in issue order (concurrent DMAs on the ring
    # round-robin and would otherwise all complete together at the end).
    q_dup = bigbuf.tile([128, EX, H2], BF16)        # q on both 64-partition halves
    c_nat = bigbuf.tile([128, EX, CH, H2], BF16)   # p = i%128
    chain = []

    def swdge(dst, src):
        ld = nc.gpsimd.dma_start(dst, src)
        if len(chain) >= 2:
            add_dep_helper(ld.ins, chain[-2].ins, sync=True, reason="load chain")
        chain.append(ld)

    swdge(q_dup[0:64, :, :], fq[:, :, :].rearrange("e j d -> j e d"))
    for e in range(EX):
        swdge(
            c_nat[:, e, :, :],
            fd[e:e + 1, :, :].rearrange("e (ch p) d -> p (e ch) d", p=128),
        )
    # wlab: strided SWDGE load (8B granules), only needed at the very end
    wlab_sb = consts.tile([128, NK, NL], F32)  # chunk k = piece*DH+dh
    swdge(wlab_sb[:, :, :], wlab[:, :].rearrange("(k p) l -> p k l", p=128))
    # duplicate q onto the upper partition half via the idle sync HWDGE
    # queue (SBUF->SBUF), keeping 0.26 MB off the critical SWDGE stream
    nc.sync.dma_start(q_dup[64:128, :, :], q_dup[0:64, :, :])

    # ---- weights: wsim/blab on the sync HWDGE queue (keeps gpsimd free) ----
    wsim_flat = consts.tile([1, 3 * H2], F32)
    nc.sync.dma_start(wsim_flat[0:1, :], wsim[:].rearrange("(o x) -> o x", o=1))
    b_sb = consts.tile([1, NL], F32)
    nc.sync.dma_start(b_sb[0:1, :], blab[:].rearrange("(o l) -> o l", o=1))

    # ---- small constants ----
    ones_bf = consts.tile([1, 128], BF16)      # K=1 broadcast lhsT
    nc.vector.memset(ones_bf[0:1, :], 1.0)
    ones128_bf = consts.tile([128, 1], BF16)   # partition-sum lhsT
    nc.vector.memset(ones128_bf[:, :], 1.0)
    ones_f32 = consts.tile([1, 128], F32)      # broadcast lhsT + [1,1] identity
    nc.vector.memset(ones_f32[0:1, :], 1.0)
    id_bf = consts.tile([128, 128], BF16)      # identity for PE transposes
    make_identity(nc, id_bf[:, :])

    # ---- persistent SBUF state ----
    c_T = bigbuf.tile([128, EX, DH, C], BF16)
    q_T = sbp.tile([128, EX, DH, Q], BF16)
    rhs_qm = sbp.tile([128, EX, DH, Q], BF16)
    qwrow = sbp.tile([1, EX * Q], BF16)
    w_sb = consts.tile([128, 6], F32)          # col = t*2+dh; t: 0=w_c 1=w_q 2=w_m
    wq_bf = consts.tile([128, DH], BF16)
    P_all = sbp.tile([128, CH, EX, Q], BF16)
    Pn_all = sbp.tile([128, CH, EX, Q], BF16)
    PT_all = sbp.tile([128, EX // 2, CH, 128], BF16)
    pm_col = sbp.tile([128, EX * CH], BF16)    # col = e*CH+ch
    den_all = sbp.tile([128, CH, EX], F32)
    rden_all = sbp.tile([128, CH, EX], F32)
    # fold-chain results: chain 0=cmax, 1=max c2q, 2=max c*c2q, 3=cmin
    stk_red = sbp.tile([128, 4, EX, DH], F32)
    p3_f = sbp.tile([128, EX, DH], F32)        # max c*q2c
    q2cT_sb = sbp.tile([128, EX * DH], F32)    # unnormalized q2c^T columns
    q2cr_sb = sbp.tile([128, EX, DH], F32)
    sumb = sbp.tile([1, EX], F32)
    recipb = sbp.tile([1, EX], F32)
    r_sb = sbp.tile([128, EX], F32)
    out_sb = sbp.tile([EX, NL], F32)

    def pe_group(dsts, srcs, f32_id=False):
        """One PSUM transpose accumulation group (start first, stop last)."""
        ident = ones_f32 if f32_id else id_bf
        first = None
        n = len(srcs)
        for k, src in enumerate(srcs):
            mm = nc.tensor.matmul(
                dsts[k], src, ident[0:src.shape[0], 0:src.shape[0]],
                is_transpose=True,
                start=(first is None), stop=(k == n - 1),
                skip_group_check=True,
            )
            if first is None:
                first = mm
            else:
                add_dep_helper(mm.ins, first.ins, sync=False, reason="bank order")
        return first

    # ---- w_sb: transpose wsim_flat [1,768] into columns [128, 6] ----
    w_ps = ps_s.tile([128, 512], F32, tag="s")
    pe_group(
        [w_ps[:, t:t + 1] for t in range(6)],
        [wsim_flat[0:1, t * 128:(t + 1) * 128] for t in range(6)],
        f32_id=True,
    )
    nc.scalar.copy(w_sb[:, :], w_ps[:, 0:6])
    nc.vector.tensor_copy(wq_bf[:, :], w_sb[:, 2:4])

    # ---- q^T for all examples + rhs_qm + qw rows ----
    for half in range(2):
        tp = ps_ct.tile([128, DH, CH, 128], BF16, tag="ct")
        tpv = tp[:, :, :, :].rearrange("p a b x -> p (a b x)")
        srcs = []
        for e in range(half * 4, half * 4 + 4):
            for dh in range(DH):
                srcs.append(q_dup[0:64, e, dh * 128:(dh + 1) * 128])
        pe_group([tpv[:, k * Q:(k + 1) * Q] for k in range(8)], srcs)
        nc.scalar.copy(
            q_T[:, half * 4:half * 4 + 4, :, :],
            tpv[:, 0:8 * Q].rearrange("p (e dh j) -> p e dh j", dh=DH, j=Q),
        )
    for dh in range(DH):
        nc.scalar.activation(
            rhs_qm[:, :, dh, :], q_T[:, :, dh, :],
            AF.Identity,
            bias=w_sb[:, 0 + dh:1 + dh], scale=w_sb[:, 4 + dh:5 + dh],
        )
    qw_ps = ps_s.tile([128, 512], F32, tag="s")
    first = None
    for e in range(EX):
        for dh in range(DH):
            mm = nc.tensor.matmul(
                qw_ps[0:1, e * Q:(e + 1) * Q], wq_bf[:, dh:dh + 1], q_T[:, e, dh, :],
                start=(dh == 0), stop=(dh == DH - 1),
                skip_group_check=True,
            )
            if first is None:
                first = mm
            else:
                add_dep_helper(mm.ins, first.ins, sync=False, reason="bank order")
    nc.vector.tensor_copy(qwrow[0:1, :], qw_ps[0:1, 0:EX * Q])

    # ---------- pipelined per-pair stages ----------
    def stage_T(p):
        """c^T transposes for pair p (PE) + per-example ACT evacuation."""
        for e in (2 * p, 2 * p + 1):
            tp = ps_ct.tile([128, DH, CH, 128], BF16, tag="ct")
            for dh in range(DH):
                pe_group(
                    [tp[:, dh, chk, :] for chk in range(CH)],
                    [c_nat[:, e, chk, dh * 128:(dh + 1) * 128] for chk in range(CH)],
                )
            nc.scalar.copy(
                c_T[:, e, :, :],
                tp[:, :, :, :].rearrange("p dh ch x -> p dh (ch x)"),
            )

    def stage_S(p):
        """S matmuls (PE) -> exp (ACT) -> den/pm/recip/Pn (DVE)."""
        e0 = 2 * p
        ps = ps_s.tile([128, CH, 2, Q], F32, tag="s")
        first = None
        for slot in range(2):
            e = e0 + slot
            for chk in range(CH):
                for dh in range(DH):
                    mm = nc.tensor.matmul(
                        ps[:, chk, slot, :],
                        c_T[:, e, dh, chk * 128:(chk + 1) * 128],
                        rhs_qm[:, e, dh, :],
                        start=(first is None), stop=False,
                        skip_group_check=True,
                    )
                    if first is None:
                        first = mm
                    else:
                        add_dep_helper(mm.ins, first.ins, sync=False, reason="bank")
                mm = nc.tensor.matmul(
                    ps[:, chk, slot, :], ones_bf[0:1, :],
                    qwrow[0:1, e * Q:(e + 1) * Q],
                    start=False, stop=(slot == 1 and chk == CH - 1),
                    skip_group_check=True,
                )
                add_dep_helper(mm.ins, first.ins, sync=False, reason="bank")

        pview = P_all[:, :, e0:e0 + 2, :]
        nc.scalar.activation(pview, ps[:, :, :, :], AF.Exp)
        den = den_all[:, :, e0:e0 + 2]
        nc.vector.reduce_sum(den, pview, axis=AX.X)
        nc.vector.tensor_reduce(
            pm_col[:, e0 * CH:(e0 + 2) * CH].rearrange("p (e c) -> p c e", c=CH),
            pview, axis=AX.X, op=OP.max,
        )
        rden = rden_all[:, :, e0:e0 + 2]
        nc.vector.reciprocal(rden, den)
        nc.vector.tensor_tensor(
            Pn_all[:, :, e0:e0 + 2, :], pview,
            rden.unsqueeze(3).broadcast_to([128, CH, 2, Q]),
            op=OP.mult,
        )

    def stage_PT(p):
        """P_norm^T via PE transpose + GPSIMD evacuation (ACT is loaded)."""
        e0 = 2 * p
        tp = ps_s.tile([128, CH, 128], BF16, tag="s")
        pe_group(
            [tp[:, chk, :] for chk in range(CH)],
            [Pn_all[:, chk, e0:e0 + 2, :] for chk in range(CH)],
        )
        # GPSIMD cannot read PSUM (BIR verifier); evacuation stays on ACT
        nc.scalar.copy(PT_all[:, p, :, :], tp[:, :, :])

    def stage_C2Q(p):
        """c2q^T = q^T @ P^T per (slot, dh), ACT evacuation to bf16."""
        c2q_sb = c2_pool.tile([128, 2, DH, C], BF16, tag="c2q")
        for slot in range(2):
            e = 2 * p + slot
            for dh in range(DH):
                ps = ps_c2q.tile([128, C], F32, tag="c2q")
                nc.tensor.matmul(
                    ps[:, :],
                    q_dup[slot * 64:slot * 64 + 64, e, dh * 128:(dh + 1) * 128],
                    PT_all[slot * 64:slot * 64 + 64, p, :, :],
                    start=True, stop=True,
                    tile_position=(slot * 64, 0),
                )
                nc.scalar.copy(c2q_sb[:, slot, dh, :], ps[:, :])
        return c2q_sb

    def stage_Q2C(p):
        """q2c^T columns: lhsT=c chunk, rhs=pm column; DVE evacuation."""
        ps = ps_s.tile([128, 2, DH], F32, tag="s")
        for slot in range(2):
            e = 2 * p + slot
            for dh in range(DH):
                first = None
                for chk in range(CH):
                    mm = nc.tensor.matmul(
                        ps[:, slot, dh:dh + 1],
                        c_nat[:, e, chk, dh * 128:(dh + 1) * 128],
                        pm_col[:, e * CH + chk:e * CH + chk + 1],
                        start=(chk == 0), stop=(chk == CH - 1),
                        skip_group_check=True,
                    )
                    if first is None:
                        first = mm
                    else:
                        add_dep_helper(mm.ins, first.ins, sync=False, reason="grp")
        nc.vector.tensor_copy(
            q2cT_sb[:, 2 * p * DH:(2 * p + 2) * DH].rearrange(
                "p (s dh) -> p s dh", dh=DH),
            ps[:, :, :],
        )

    def emit_stk_reduce(p):
        """Stacked final reduce for pair p's fold chains (DVE): chains 0-2
        are max-reduced together, chain 3 (cmin) min-reduced."""
        nc.vector.tensor_reduce(
            stk_red[:, 0:3, 2 * p:2 * p + 2, :], stk3_tiles[p][:, 0:3, :, :, :],
            axis=AX.X, op=OP.max,
        )
        nc.vector.tensor_reduce(
            stk_red[:, 3:4, 2 * p:2 * p + 2, :], stk3_tiles[p][:, 3:4, :, :, :],
            axis=AX.X, op=OP.min,
        )

    stk3_tiles = {}

    def stage_FOLD(p, c2q_sb):
        """Max-pools over context: fold trees split DVE/GPSIMD, one stacked
        reduce per pair (deferred one stage so GPSIMD folds can finish)."""
        if p > 0:
            emit_stk_reduce(p - 1)
        eP = slice(2 * p, 2 * p + 2)
        # DVE: c2q fold1 + prod
        f1p1 = scr_pool.tile([128, 2, DH, 256], BF16, tag="f1p1")
        nc.vector.tensor_tensor(
            f1p1[:, :, :, :], c2q_sb[:, :, :, 0:256], c2q_sb[:, :, :, 256:512],
            op=OP.max)
        prod = scr_pool.tile([128, 2, DH, C], BF16, tag="prod")
        nc.vector.tensor_tensor(
            prod[:, :, :, :], c_T[:, eP, :, :], c2q_sb[:, :, :, :], op=OP.mult)
        # GPSIMD fold trees; chains 0=cmax 1=p1 2=p2 3=cmin
        stk3 = scr_pool.tile([128, 4, 2, DH, 64], BF16, tag="stk3")
        stk3_tiles[p] = stk3
        f1p0 = scr_pool.tile([128, 2, DH, 256], BF16, tag="f1p0")
        f1cm = scr_pool.tile([128, 2, DH, 256], BF16, tag="f1cm")
        f1p2 = scr_pool.tile([128, 2, DH, 256], BF16, tag="f1p2")
        f2 = {}
        for c in range(4):
            f2c = scr_pool.tile([128, 2, DH, 128], BF16, tag=f"f2_{c}", name=f"f2_{c}")
            f2[c] = f2c

        geng = nc.gpsimd if os.environ.get("K_GPS_FOLD", "0") == "1" else nc.vector

        def gfold23(chain, f1, op):
            geng.tensor_tensor(
                f2[chain][:, :, :, :], f1[:, :, :, 0:128], f1[:, :, :, 128:256],
                op=op)
            geng.tensor_tensor(
                stk3[:, chain, :, :, :],
                f2[chain][:, :, :, 0:64], f2[chain][:, :, :, 64:128], op=op)

        geng.tensor_tensor(
            f1p0[:, :, :, :], c_T[:, eP, :, 0:256], c_T[:, eP, :, 256:512],
            op=OP.max)
        gfold23(0, f1p0, OP.max)
        geng.tensor_tensor(
            f1cm[:, :, :, :], c_T[:, eP, :, 0:256], c_T[:, eP, :, 256:512],
            op=OP.min)
        gfold23(3, f1cm, OP.min)
        gfold23(1, f1p1, OP.max)
        geng.tensor_tensor(
            f1p2[:, :, :, :], prod[:, :, :, 0:256], prod[:, :, :, 256:512],
            op=OP.max)
        gfold23(2, f1p2, OP.max)

    # ---------- run the pipeline ----------
    # stage_S(p+1) is issued BEFORE stage_FOLD(p): the folds are off the
    # critical path (they only feed the final matmul), and the next pair's
    # softmax must not queue behind them on the DVE.
    stage_T(0)
    stage_S(0)
    for p in range(EX // 2):
        if p + 1 < EX // 2:
            stage_T(p + 1)
        stage_PT(p)
        c2q_sb = stage_C2Q(p)
        stage_Q2C(p)
        if p + 1 < EX // 2:
            stage_S(p + 1)
        stage_FOLD(p, c2q_sb)

    # ---------- epilogue: b_att sums, q2c scale, piece3, final matmul ----------
    emit_stk_reduce(EX // 2 - 1)
    bs_ps = ps_s.tile([128, 512], F32, tag="s")
    nc.tensor.matmul(
        bs_ps[0:1, 0:EX * CH], ones128_bf[:, :], pm_col[:, :],
        start=True, stop=True,
    )
    nc.vector.reduce_sum(
        sumb[0:1, :],
        bs_ps[0:1, 0:EX * CH].rearrange("o (e c) -> o e c", c=CH),
        axis=AX.X,
    )
    nc.vector.reciprocal(recipb[0:1, :], sumb[0:1, :])
    rb_ps = ps_s.tile([128, 512], F32, tag="s")
    nc.tensor.matmul(
        rb_ps[:, 0:EX], ones_f32[0:1, :], recipb[0:1, :],
        start=True, stop=True,
    )
    nc.vector.tensor_copy(r_sb[:, :], rb_ps[:, 0:EX])

    q2v = q2cT_sb[:, :].rearrange("p (e dh) -> p e dh", dh=DH)
    nc.vector.tensor_tensor(
        q2cr_sb[:, :, :],
        q2v, r_sb[:, :].unsqueeze(2).broadcast_to([128, EX, DH]), op=OP.mult)
    s3a = scr_pool.tile([128, EX, DH], F32, tag="s3a")
    s3b = scr_pool.tile([128, EX, DH], F32, tag="s3b")
    nc.vector.tensor_tensor(s3a[:, :, :], q2cr_sb[:, :, :], stk_red[:, 0, :, :], op=OP.mult)
    nc.vector.tensor_tensor(s3b[:, :, :], q2cr_sb[:, :, :], stk_red[:, 3, :, :], op=OP.mult)
    nc.vector.tensor_tensor(p3_f[:, :, :], s3a[:, :, :], s3b[:, :, :], op=OP.max)

    out_ps = ps_s.tile([128, 512], F32, tag="s")
    first = None
    for piece in range(4):
        for dh in range(DH):
            if piece < 3:
                pv = stk_red[:, piece, :, dh]
            else:
                pv = p3_f[:, :, dh]
            mm = nc.tensor.matmul(
                out_ps[0:EX, 0:NL], pv, wlab_sb[:, piece * DH + dh, :],
                start=(first is None), stop=False, skip_group_check=True,
            )
            if first is None:
                first = mm
            else:
                add_dep_helper(mm.ins, first.ins, sync=False, reason="bank")
    mm = nc.tensor.matmul(
        out_ps[0:EX, 0:NL], ones_f32[0:1, 0:EX], b_sb[0:1, :],
        start=False, stop=True, skip_group_check=True,
    )
    add_dep_helper(mm.ins, first.ins, sync=False, reason="bank")
    nc.vector.tensor_copy(out_sb[:, :], out_ps[0:EX, 0:NL])
    nc.sync.dma_start(out[:, :], out_sb[:, :])

    if os.environ.get("K_DEBUG", "0") == "1":
        dumps = [
            ("dbg_stk", stk_red[:, :, :, :].rearrange("p a e d -> p (a e d)"), 4 * EX * DH, F32),
            ("dbg_p3", p3_f[:, :, :].rearrange("p e d -> p (e d)"), EX * DH, F32),
            ("dbg_q2cr", q2cr_sb[:, :, :].rearrange("p e d -> p (e d)"), EX * DH, F32),
            ("dbg_q2cT", q2cT_sb[:, :], EX * DH, F32),
            ("dbg_r", r_sb[:, :], EX, F32),
            ("dbg_pm", pm_col[:, :], EX * CH, BF16),
            ("dbg_den", den_all[:, :, :].rearrange("p c e -> p (c e)"), CH * EX, F32),
        ]
        for name, view, n, dt in dumps:
            t = nc.dram_tensor(name, [128, n], dt, kind="ExternalOutput")
            nc.sync.dma_start(t[:, :], view)
        tq = nc.dram_tensor("dbg_qw", [1, EX * Q], BF16, kind="ExternalOutput")
        nc.sync.dma_start(tq[:, :], qwrow[0:1, :])


def build_nc():
    nc = bacc.Bacc("TRN2", target_bir_lowering=False, debug=False)
    fd = nc.dram_tensor("fd", [EX, C, H2], F32, kind="ExternalInput")
    fq = nc.dram_tensor("fq", [EX, Q, H2], F32, kind="ExternalInput")
    wsim = nc.dram_tensor("wsim", [3 * H2], F32, kind="ExternalInput")
    wlab = nc.dram_tensor("wlab", [4 * H2, NL], F32, kind="ExternalInput")
    blab = nc.dram_tensor("blab", [NL], F32, kind="ExternalInput")
    out = nc.dram_tensor("out", [EX, NL], F32, kind="ExternalOutput")

    from contextlib import ExitStack
    with tile.TileContext(nc) as tc:
        with ExitStack() as ctx:
            _body(tc, ctx, fd[:, :, :], fq[:, :, :], wsim[:], wlab[:, :], blab[:], out[:, :])
    nc.compile()
    return nc


_NC_CACHE = None


def run(inputs, trace=False):
    global _NC_CACHE
    if _NC_CACHE is None:
        _NC_CACHE = build_nc()
    nc = _NC_CACHE

    fd = np.ascontiguousarray(np.asarray(inputs["feature_document"], dtype=np.float32))
    fq = np.ascontiguousarray(np.asarray(inputs["feature_query"], dtype=np.float32))
    wsim = np.ascontiguousarray(np.asarray(inputs["w_sim"], dtype=np.float32))
    wlab = np.ascontiguousarray(np.asarray(inputs["w_label"], dtype=np.float32))
    blab = np.ascontiguousarray(np.asarray(inputs["b_label"], dtype=np.float32))

    in_maps = []
    for core in range(N_CORES):
        sl = slice(core * EX, (core + 1) * EX)
        in_maps.append({
            "fd": fd[sl], "fq": fq[sl],
            "wsim": wsim, "wlab": wlab, "blab": blab,
        })
    res = run_bass_kernel_spmd(nc, in_maps, list(range(N_CORES)), trace=trace)
    outs = np.concatenate([np.asarray(res.results[i]["out"]) for i in range(N_CORES)], axis=0)
    return outs.astype(np.float32), res


def kernel(**inputs):
    outs, _ = run(inputs, trace=False)
    return outs


# revision 33
# speedup vs baseline: 1.2632x; 1.0299x over previous
"""BiDAF attention-flow kernel for Trainium2 (8 NeuronCores, data-parallel).

Self-contained: hardcodes shapes B,C,Q,H2 = 64,512,64,256; n_labels=2.
kernel(**inputs) takes full unsharded inputs, shards batch over 8 cores,
runs one SPMD Bass/Tile kernel, gathers [8,2] per core -> [64,2].

Per-core math (8 examples, bf16 compute, fp32 accumulation):
  S = c @ diag(w_m) @ q^T + (c@w_c)[:,None] + (q@w_q)[None,:]
    - the c@w_c term folds into the matmul rhs (rhs = w_m*q^T + w_c),
    - the q@w_q term rides in via a K=1 all-ones broadcast matmul.
  P = exp(S) unstabilized (|S| is O(1) for this distribution); row-softmax
  needs only row sums, and b_att = softmax(max_j S) = Pmax/sum(Pmax) with
  Pmax = max_j P (exp is monotone).

Structure vs the previous version:
  - c loads: one SWDGE cast-DMA per example, chained with lookahead-2
    (two DMAs in flight: no inter-link bubbles, staggered completion);
    weights ride the sync-queue HWDGE (wsim flat + on-chip PE transpose),
    wlab issued last (needed at end); q duplicated via sync SBUF copy.
  - max-pools as fold trees split across DVE and GPSIMD (GPSIMD is idle
    during compute), with one stacked tensor_reduce per pair.
  - q2c flipped: lhsT=c chunk, rhs=pm column -> q2c^T lands as a PSUM
    column in the right (d-major) layout, no epilogue transposes.
  - software-pipelined issue order: pair p+1's c^T transposes are issued
    between S(p) and P^T(p) so the PE stays busy during softmax latency;
    next pair's softmax is issued before this pair's folds on the DVE.
"""

import os
import sys

for _p in ("/opt/trn_rl_repo", "/opt/pypackages"):
    if os.path.isdir(_p) and _p not in sys.path:
        sys.path.insert(0, _p)

import numpy as np

import concourse.bass as bass
import concourse.bacc as bacc
import concourse.tile as tile
import concourse.mybir as mybir
from concourse.bass_utils import run_bass_kernel_spmd
from concourse.masks import make_identity
from concourse.tile_rust import add_dep_helper

F32 = mybir.dt.float32
BF16 = mybir.dt.bfloat16
AX = mybir.AxisListType
OP = mybir.AluOpType
AF = mybir.ActivationFunctionType

N_CORES = 8
B, C, Q, H2 = 64, 512, 64, 256
NL = 2
EX = B // N_CORES          # examples per core = 8
CH = C // 128              # context chunks of 128 = 4
DH = H2 // 128             # feature chunks of 128 = 2
NK = 4 * DH                # final feature chunks (4 pieces x DH) = 8
NEG = -1e30
POS = 1e30

USE_TTR = int(os.environ.get("K_USE_TTR", "0"))  # 0=fold trees, 1=ttr max, 2=ttr min(-x)
CHAIN_LOADS = os.environ.get("K_CHAIN_LOADS", "0") == "1"


def _body(tc, ctx, fd, fq, wsim, wlab, blab, out):
    nc = tc.nc

    consts = ctx.enter_context(tc.tile_pool(name="consts", bufs=1))
    bigbuf = ctx.enter_context(tc.tile_pool(name="bigbuf", bufs=1))
    sbp = ctx.enter_context(tc.tile_pool(name="sbp", bufs=1))
    c2_pool = ctx.enter_context(tc.tile_pool(name="c2", bufs=2))
    scr_pool = ctx.enter_context(tc.tile_pool(name="scr", bufs=2))

    ps_ct = ctx.enter_context(tc.tile_pool(name="psct", bufs=2, space="PSUM"))
    ps_s = ctx.enter_context(tc.tile_pool(name="pss", bufs=2, space="PSUM"))
    ps_c2q = ctx.enter_context(tc.tile_pool(name="psc2q", bufs=2, space="PSUM"))

    # ---- big inputs FIRST: cast-load fp32 -> bf16 (SWDGE), unchained per
    # example.  All loads share SWDGE ring 0; each engine drains its ring
    # FIFO, so earlier dma_starts complete first and compute streams behind
    # the loads.  Issued before everything else so descriptor gen starts at
    # preamble end.
    # Chain with lookahead-2: dma k waits on dma k-2, so exactly two DMAs
    # share the SDMA ring at any time -> no inter-link bubbles, but
    # completion still staggers in issue order (concurrent DMAs on the ring
    # round-robin and would otherwise all complete together at the end).
    q_dup = bigbuf.tile([128, EX, H2], BF16)        # q on both 64-partition halves
    c_nat = bigbuf.tile([128, EX, CH, H2], BF16)   # p = i%128
    chain = []

    def swdge(dst, src):
        ld = nc.gpsimd.dma_start(dst, src)
        if len(chain) >= 2:
            add_dep_helper(ld.ins, chain[-2].ins, sync=True, reason="load chain")
        chain.append(ld)

    swdge(q_dup[0:64, :, :], fq[:, :, :].rearrange("e j d -> j e d"))
    for e in range(EX):
        swdge(
            c_nat[:, e, :, :],
            fd[e:e + 1, :, :].rearrange("e (ch p) d -> p (e ch) d", p=128),
        )
    # wlab: strided SWDGE load (8B granules), only needed at the very end
    wlab_sb = consts.tile([128, NK, NL], F32)  # chunk k = piece*DH+dh
    swdge(wlab_sb[:, :, :], wlab[:, :].rearrange("(k p) l -> p k l", p=128))
    # duplicate q onto the upper partition half via the idle sync HWDGE
    # queue (SBUF->SBUF), keeping 0.26 MB off the critical SWDGE stream
    nc.sync.dma_start(q_dup[64:128, :, :], q_dup[0:64, :, :])

    # ---- weights: wsim/blab on the sync HWDGE queue (keeps gpsimd free) ----
    wsim_flat = consts.tile([1, 3 * H2], F32)
    nc.sync.dma_start(wsim_flat[0:1, :], wsim[:].rearrange("(o x) -> o x", o=1))
    b_sb = consts.tile([1, NL], F32)
    nc.sync.dma_start(b_sb[0:1, :], blab[:].rearrange("(o l) -> o l", o=1))

    # ---- small constants ----
    ones_bf = consts.tile([1, 128], BF16)      # K=1 broadcast lhsT
    nc.vector.memset(ones_bf[0:1, :], 1.0)
    ones128_bf = consts.tile([128, 1], BF16)   # partition-sum lhsT
    nc.vector.memset(ones128_bf[:, :], 1.0)
    ones_f32 = consts.tile([1, 128], F32)      # broadcast lhsT + [1,1] identity
    nc.vector.memset(ones_f32[0:1, :], 1.0)
    id_bf = consts.tile([128, 128], BF16)      # identity for PE transposes
    make_identity(nc, id_bf[:, :])

    # ---- persistent SBUF state ----
    c_T = bigbuf.tile([128, EX, DH, C], BF16)
    q_T = sbp.tile([128, EX, DH, Q], BF16)
    rhs_qm = sbp.tile([128, EX, DH, Q], BF16)
    qwrow = sbp.tile([1, EX * Q], BF16)
    w_sb = consts.tile([128, 6], F32)          # col = t*2+dh; t: 0=w_c 1=w_q 2=w_m
    wq_bf = consts.tile([128, DH], BF16)
    P_all = sbp.tile([128, CH, EX, Q], BF16)
    Pn_all = sbp.tile([128, CH, EX, Q], BF16)
    PT_all = sbp.tile([128, EX // 2, CH, 128], BF16)
    pm_col = sbp.tile([128, EX * CH], BF16)    # col = e*CH+ch
    den_all = sbp.tile([128, CH, EX], F32)
    rden_all = sbp.tile([128, CH, EX], F32)
    # fold-chain results: chain 0=cmax, 1=max c2q, 2=max c*c2q, 3=cmin
    stk_red = sbp.tile([128, 4, EX, DH], F32)
    p3_f = sbp.tile([128, EX, DH], F32)        # max c*q2c
    q2cT_sb = sbp.tile([128, EX * DH], F32)    # unnormalized q2c^T columns
    q2cr_sb = sbp.tile([128, EX, DH], F32)
    sumb = sbp.tile([1, EX], F32)
    recipb = sbp.tile([1, EX], F32)
    r_sb = sbp.tile([128, EX], F32)
    out_sb = sbp.tile([EX, NL], F32)

    def pe_group(dsts, srcs, f32_id=False):
        """One PSUM transpose accumulation group (start first, stop last)."""
        ident = ones_f32 if f32_id else id_bf
        first = None
        n = len(srcs)
        for k, src in enumerate(srcs):
            mm = nc.tensor.matmul(
                dsts[k], src, ident[0:src.shape[0], 0:src.shape[0]],
                is_transpose=True,
                start=(first is None), stop=(k == n - 1),
                skip_group_check=True,
            )
            if first is None:
                first = mm
            else:
                add_dep_helper(mm.ins, first.ins, sync=False, reason="bank order")
        return first

    # ---- w_sb: transpose wsim_flat [1,768] into columns [128, 6] ----
    w_ps = ps_s.tile([128, 512], F32, tag="s")
    pe_group(
        [w_ps[:, t:t + 1] for t in range(6)],
        [wsim_flat[0:1, t * 128:(t + 1) * 128] for t in range(6)],
        f32_id=True,
    )
    nc.scalar.copy(w_sb[:, :], w_ps[:, 0:6])
    nc.vector.tensor_copy(wq_bf[:, :], w_sb[:, 2:4])

    # ---- q^T for all examples + rhs_qm + qw rows ----
    for half in range(2):
        tp = ps_ct.tile([128, DH, CH, 128], BF16, tag="ct")
        tpv = tp[:, :, :, :].rearrange("p a b x -> p (a b x)")
        srcs = []
        for e in range(half * 4, half * 4 + 4):
            for dh in range(DH):
                srcs.append(q_dup[0:64, e, dh * 128:(dh + 1) * 128])
        pe_group([tpv[:, k * Q:(k + 1) * Q] for k in range(8)], srcs)
        nc.scalar.copy(
            q_T[:, half * 4:half * 4 + 4, :, :],
            tpv[:, 0:8 * Q].rearrange("p (e dh j) -> p e dh j", dh=DH, j=Q),
        )
    for dh in range(DH):
        nc.scalar.activation(
            rhs_qm[:, :, dh, :], q_T[:, :, dh, :],
            AF.Identity,
            bias=w_sb[:, 0 + dh:1 + dh], scale=w_sb[:, 4 + dh:5 + dh],
        )
    qw_ps = ps_s.tile([128, 512], F32, tag="s")
    first = None
    for e in range(EX):
        for dh in range(DH):
            mm = nc.tensor.matmul(
                qw_ps[0:1, e * Q:(e + 1) * Q], wq_bf[:, dh:dh + 1], q_T[:, e, dh, :],
                start=(dh == 0), stop=(dh == DH - 1),
                skip_group_check=True,
            )
            if first is None:
                first = mm
            else:
                add_dep_helper(mm.ins, first.ins, sync=False, reason="bank order")
    nc.vector.tensor_copy(qwrow[0:1, :], qw_ps[0:1, 0:EX * Q])

    # ---------- pipelined per-pair stages ----------
    def stage_T(p):
        """c^T transposes for pair p (PE) + per-example ACT evacuation."""
        for e in (2 * p, 2 * p + 1):
            tp = ps_ct.tile([128, DH, CH, 128], BF16, tag="ct")
            for dh in range(DH):
                pe_group(
                    [tp[:, dh, chk, :] for chk in range(CH)],
                    [c_nat[:, e, chk, dh * 128:(dh + 1) * 128] for chk in range(CH)],
                )
            nc.scalar.copy(
                c_T[:, e, :, :],
                tp[:, :, :, :].rearrange("p dh ch x -> p dh (ch x)"),
            )

    def stage_S_mm(p):
        """S matmuls (PE) -> exp (ACT)."""
        e0 = 2 * p
        ps = ps_s.tile([128, CH, 2, Q], F32, tag="s")
        first = None
        for slot in range(2):
            e = e0 + slot
            for chk in range(CH):
                for dh in range(DH):
                    mm = nc.tensor.matmul(
                        ps[:, chk, slot, :],
                        c_T[:, e, dh, chk * 128:(chk + 1) * 128],
                        rhs_qm[:, e, dh, :],
                        start=(first is None), stop=False,
                        skip_group_check=True,
                    )
                    if first is None:
                        first = mm
                    else:
                        add_dep_helper(mm.ins, first.ins, sync=False, reason="bank")
                mm = nc.tensor.matmul(
                    ps[:, chk, slot, :], ones_bf[0:1, :],
                    qwrow[0:1, e * Q:(e + 1) * Q],
                    start=False, stop=(slot == 1 and chk == CH - 1),
                    skip_group_check=True,
                )
                add_dep_helper(mm.ins, first.ins, sync=False, reason="bank")
        pview = P_all[:, :, e0:e0 + 2, :]
        nc.scalar.activation(pview, ps[:, :, :, :], AF.Exp)

    def stage_S_dve(p):
        """Softmax sums/normalize (DVE), issued after ready fold work."""
        e0 = 2 * p
        pview = P_all[:, :, e0:e0 + 2, :]
        den = den_all[:, :, e0:e0 + 2]
        nc.vector.reduce_sum(den, pview, axis=AX.X)
        nc.vector.tensor_reduce(
            pm_col[:, e0 * CH:(e0 + 2) * CH].rearrange("p (e c) -> p c e", c=CH),
            pview, axis=AX.X, op=OP.max,
        )
        rden = rden_all[:, :, e0:e0 + 2]
        nc.vector.reciprocal(rden, den)
        nc.vector.tensor_tensor(
            Pn_all[:, :, e0:e0 + 2, :], pview,
            rden.unsqueeze(3).broadcast_to([128, CH, 2, Q]),
            op=OP.mult,
        )

    def stage_PT(p):
        """P_norm^T via PE transpose + GPSIMD evacuation (ACT is loaded)."""
        e0 = 2 * p
        tp = ps_s.tile([128, CH, 128], BF16, tag="s")
        pe_group(
            [tp[:, chk, :] for chk in range(CH)],
            [Pn_all[:, chk, e0:e0 + 2, :] for chk in range(CH)],
        )
        # GPSIMD cannot read PSUM (BIR verifier); evacuation stays on ACT
        nc.scalar.copy(PT_all[:, p, :, :], tp[:, :, :])

    def stage_C2Q(p):
        """c2q^T = q^T @ P^T per (slot, dh), ACT evacuation to bf16."""
        c2q_sb = c2_pool.tile([128, 2, DH, C], BF16, tag="c2q")
        for slot in range(2):
            e = 2 * p + slot
            for dh in range(DH):
                ps = ps_c2q.tile([128, C], F32, tag="c2q")
                nc.tensor.matmul(
                    ps[:, :],
                    q_dup[slot * 64:slot * 64 + 64, e, dh * 128:(dh + 1) * 128],
                    PT_all[slot * 64:slot * 64 + 64, p, :, :],
                    start=True, stop=True,
                    tile_position=(slot * 64, 0),
                )
                nc.scalar.copy(c2q_sb[:, slot, dh, :], ps[:, :])
        return c2q_sb

    def stage_Q2C(p):
        """q2c^T columns: lhsT=c chunk, rhs=pm column; DVE evacuation."""
        ps = ps_s.tile([128, 2, DH], F32, tag="s")
        for slot in range(2):
            e = 2 * p + slot
            for dh in range(DH):
                first = None
                for chk in range(CH):
                    mm = nc.tensor.matmul(
                        ps[:, slot, dh:dh + 1],
                        c_nat[:, e, chk, dh * 128:(dh + 1) * 128],
                        pm_col[:, e * CH + chk:e * CH + chk + 1],
                        start=(chk == 0), stop=(chk == CH - 1),
                        skip_group_check=True,
                    )
                    if first is None:
                        first = mm
                    else:
                        add_dep_helper(mm.ins, first.ins, sync=False, reason="grp")
        nc.vector.tensor_copy(
            q2cT_sb[:, 2 * p * DH:(2 * p + 2) * DH].rearrange(
                "p (s dh) -> p s dh", dh=DH),
            ps[:, :, :],
        )

    def emit_stk_reduce(p):
        """Stacked final reduce for pair p's fold chains (DVE): chains 0-2
        are max-reduced together, chain 3 (cmin) min-reduced."""
        nc.vector.tensor_reduce(
            stk_red[:, 0:3, 2 * p:2 * p + 2, :], stk3_tiles[p][:, 0:3, :, :, :],
            axis=AX.X, op=OP.max,
        )
        nc.vector.tensor_reduce(
            stk_red[:, 3:4, 2 * p:2 * p + 2, :], stk3_tiles[p][:, 3:4, :, :, :],
            axis=AX.X, op=OP.min,
        )

    stk3_tiles = {}
    fold_state = {}

    def gfold23(p, chain, f1, op):
        nc.vector.tensor_tensor(
            fold_state[p][chain][:, :, :, :],
            f1[:, :, :, 0:128], f1[:, :, :, 128:256], op=op)
        nc.vector.tensor_tensor(
            stk3_tiles[p][:, chain, :, :, :],
            fold_state[p][chain][:, :, :, 0:64],
            fold_state[p][chain][:, :, :, 64:128], op=op)

    def stage_FOLD_early(p):
        """cmax/cmin fold chains: depend only on c_T(p), so they fill the
        DVE while the next pair's exp cooks on the ACT."""
        eP = slice(2 * p, 2 * p + 2)
        stk3 = scr_pool.tile([128, 4, 2, DH, 64], BF16, tag="stk3")
        stk3_tiles[p] = stk3
        f2 = {}
        for c in range(4):
            f2c = scr_pool.tile([128, 2, DH, 128], BF16, tag=f"f2_{c}", name=f"f2_{c}")
            f2[c] = f2c
        fold_state[p] = f2
        f1p0 = scr_pool.tile([128, 2, DH, 256], BF16, tag="f1p0")
        nc.vector.tensor_tensor(
            f1p0[:, :, :, :], c_T[:, eP, :, 0:256], c_T[:, eP, :, 256:512],
            op=OP.max)
        gfold23(p, 0, f1p0, OP.max)
        f1cm = scr_pool.tile([128, 2, DH, 256], BF16, tag="f1cm")
        nc.vector.tensor_tensor(
            f1cm[:, :, :, :], c_T[:, eP, :, 0:256], c_T[:, eP, :, 256:512],
            op=OP.min)
        gfold23(p, 3, f1cm, OP.min)

    def stage_FOLD_late(p, c2q_sb):
        """c2q-dependent fold chains + deferred stacked reduce of pair p-1."""
        if p > 0:
            emit_stk_reduce(p - 1)
        eP = slice(2 * p, 2 * p + 2)
        f1p1 = scr_pool.tile([128, 2, DH, 256], BF16, tag="f1p1")
        nc.vector.tensor_tensor(
            f1p1[:, :, :, :], c2q_sb[:, :, :, 0:256], c2q_sb[:, :, :, 256:512],
            op=OP.max)
        prod = scr_pool.tile([128, 2, DH, C], BF16, tag="prod")
        nc.vector.tensor_tensor(
            prod[:, :, :, :], c_T[:, eP, :, :], c2q_sb[:, :, :, :], op=OP.mult)
        gfold23(p, 1, f1p1, OP.max)
        f1p2 = scr_pool.tile([128, 2, DH, 256], BF16, tag="f1p2")
        nc.vector.tensor_tensor(
            f1p2[:, :, :, :], prod[:, :, :, 0:256], prod[:, :, :, 256:512],
            op=OP.max)
        gfold23(p, 2, f1p2, OP.max)

    # ---------- run the pipeline ----------
    # Issue-order pipelining: pair p+1's transposes and S matmuls are issued
    # on the PE right after PT(p) so exp(p+1) lands early; on the DVE the
    # load-independent cmax/cmin folds of pair p run BEFORE pair p+1's
    # softmax (which waits on exp), keeping the DVE queue from idling.
    stage_T(0)
    stage_S_mm(0)
    stage_S_dve(0)
    for p in range(EX // 2):
        if p + 1 < EX // 2:
            stage_T(p + 1)
        stage_PT(p)
        if p + 1 < EX // 2:
            stage_S_mm(p + 1)
        stage_FOLD_early(p)
        c2q_sb = stage_C2Q(p)
        stage_Q2C(p)
        if p + 1 < EX // 2:
            stage_S_dve(p + 1)
        stage_FOLD_late(p, c2q_sb)

    # ---------- epilogue: b_att sums, q2c scale, piece3, final matmul ----------
    emit_stk_reduce(EX // 2 - 1)
    bs_ps = ps_s.tile([128, 512], F32, tag="s")
    nc.tensor.matmul(
        bs_ps[0:1, 0:EX * CH], ones128_bf[:, :], pm_col[:, :],
        start=True, stop=True,
    )
    nc.vector.reduce_sum(
        sumb[0:1, :],
        bs_ps[0:1, 0:EX * CH].rearrange("o (e c) -> o e c", c=CH),
        axis=AX.X,
    )
    nc.vector.reciprocal(recipb[0:1, :], sumb[0:1, :])
    rb_ps = ps_s.tile([128, 512], F32, tag="s")
    nc.tensor.matmul(
        rb_ps[:, 0:EX], ones_f32[0:1, :], recipb[0:1, :],
        start=True, stop=True,
    )
    nc.vector.tensor_copy(r_sb[:, :], rb_ps[:, 0:EX])

    q2v = q2cT_sb[:, :].rearrange("p (e dh) -> p e dh", dh=DH)
    nc.vector.tensor_tensor(
        q2cr_sb[:, :, :],
        q2v, r_sb[:, :].unsqueeze(2).broadcast_to([128, EX, DH]), op=OP.mult)
    s3a = scr_pool.tile([128, EX, DH], F32, tag="s3a")
    s3b = scr_pool.tile([128, EX, DH], F32, tag="s3b")
    nc.vector.tensor_tensor(s3a[:, :, :], q2cr_sb[:, :, :], stk_red[:, 0, :, :], op=OP.mult)
    nc.vector.tensor_tensor(s3b[:, :, :], q2cr_sb[:, :, :], stk_red[:, 3, :, :], op=OP.mult)
    nc.vector.tensor_tensor(p3_f[:, :, :], s3a[:, :, :], s3b[:, :, :], op=OP.max)

    out_ps = ps_s.tile([128, 512], F32, tag="s")
    first = None
    for piece in range(4):
        for dh in range(DH):
            if piece < 3:
                pv = stk_red[:, piece, :, dh]
            else:
                pv = p3_f[:, :, dh]
            mm = nc.tensor.matmul(
                out_ps[0:EX, 0:NL], pv, wlab_sb[:, piece * DH + dh, :],
                start=(first is None), stop=False, skip_group_check=True,
            )
            if first is None:
                first = mm
            else:
                add_dep_helper(mm.ins, first.ins, sync=False, reason="bank")
    mm = nc.tensor.matmul(
        out_ps[0:EX, 0:NL], ones_f32[0:1, 0:EX], b_sb[0:1, :],
        start=False, stop=True, skip_group_check=True,
    )
    add_dep_helper(mm.ins, first.ins, sync=False, reason="bank")
    nc.vector.tensor_copy(out_sb[:, :], out_ps[0:EX, 0:NL])
    nc.sync.dma_start(out[:, :], out_sb[:, :])

    if os.environ.get("K_DEBUG", "0") == "1":
        dumps = [
            ("dbg_stk", stk_red[:, :, :, :].rearrange("p a e d -> p (a e d)"), 4 * EX * DH, F32),
            ("dbg_p3", p3_f[:, :, :].rearrange("p e d -> p (e d)"), EX * DH, F32),
            ("dbg_q2cr", q2cr_sb[:, :, :].rearrange("p e d -> p (e d)"), EX * DH, F32),
            ("dbg_q2cT", q2cT_sb[:, :], EX * DH, F32),
            ("dbg_r", r_sb[:, :], EX, F32),
            ("dbg_pm", pm_col[:, :], EX * CH, BF16),
            ("dbg_den", den_all[:, :, :].rearrange("p c e -> p (c e)"), CH * EX, F32),
        ]
        for name, view, n, dt in dumps:
            t = nc.dram_tensor(name, [128, n], dt, kind="ExternalOutput")
            nc.sync.dma_start(t[:, :], view)
        tq = nc.dram_tensor("dbg_qw", [1, EX * Q], BF16, kind="ExternalOutput")
        nc.sync.dma_start(tq[:, :], qwrow[0:1, :])


def build_nc():
    nc = bacc.Bacc("TRN2", target_bir_lowering=False, debug=False)
    fd = nc.dram_tensor("fd", [EX, C, H2], F32, kind="ExternalInput")
    fq = nc.dram_tensor("fq", [EX, Q, H2], F32, kind="ExternalInput")
    wsim = nc.dram_tensor("wsim", [3 * H2], F32, kind="ExternalInput")
    wlab = nc.dram_tensor("wlab", [4 * H2, NL], F32, kind="ExternalInput")
    blab = nc.dram_tensor("blab", [NL], F32, kind="ExternalInput")
    out = nc.dram_tensor("out", [EX, NL], F32, kind="ExternalOutput")

    from contextlib import ExitStack
    with tile.TileContext(nc) as tc:
        with ExitStack() as ctx:
            _body(tc, ctx, fd[:, :, :], fq[:, :, :], wsim[:], wlab[:, :], blab[:], out[:, :])
    nc.compile()
    return nc


_NC_CACHE = None


def run(inputs, trace=False):
    global _NC_CACHE
    if _NC_CACHE is None:
        _NC_CACHE = build_nc()
    nc = _NC_CACHE

    fd = np.ascontiguousarray(np.asarray(inputs["feature_document"], dtype=np.float32))
    fq = np.ascontiguousarray(np.asarray(inputs["feature_query"], dtype=np.float32))
    wsim = np.ascontiguousarray(np.asarray(inputs["w_sim"], dtype=np.float32))
    wlab = np.ascontiguousarray(np.asarray(inputs["w_label"], dtype=np.float32))
    blab = np.ascontiguousarray(np.asarray(inputs["b_label"], dtype=np.float32))

    in_maps = []
    for core in range(N_CORES):
        sl = slice(core * EX, (core + 1) * EX)
        in_maps.append({
            "fd": fd[sl], "fq": fq[sl],
            "wsim": wsim, "wlab": wlab, "blab": blab,
        })
    res = run_bass_kernel_spmd(nc, in_maps, list(range(N_CORES)), trace=trace)
    outs = np.concatenate([np.asarray(res.results[i]["out"]) for i in range(N_CORES)], axis=0)
    return outs.astype(np.float32), res


def kernel(**inputs):
    outs, _ = run(inputs, trace=False)
    return outs
